# revision 33
# baseline (speedup 1.0000x reference)
"""Trainium2 Bass kernel for nn_Attention_4380866642117.

Math: the reference computes additive-score attention
    score[b,i,j] = q[b,i].w_q + k[b,j].w_k ; masked (mask==True -> -1e10)
    attn = softmax_j(score); out = LN(attn @ v @ fc_w.T + q)
The additive score's q-term is constant along the softmax axis and cancels;
masked logits (-1e10) underflow to exactly 0 in the f32 softmax.  With
e_j = exp(k[b,j].w_k):
    attn[b,i,j] = (1-m[b,i,j]) * e_j / Z_i,  Z_i = sum_j (1-m) e_j
    out[b,i,:]  = LN( (1/Z_i) sum_j (1-m) e_j (v@fc_w.T)[j,:] + q[b,i,:] )
so attention+fc collapse to one masked matmul against fc-pretransformed,
e-weighted values, plus a rank-1 masked elementwise product for attn.

Distribution: data-parallel over batch (8 batches -> 8 NeuronCores).

The mask enters the PE with j on partitions (contraction dim) by loading it
through the DMA xbar transpose viewing byte pairs as one fp16 element; the
resulting row pairing j = 256c + 2p + b is absorbed by loading k/v rows with
the same permuted access pattern (contraction is order-invariant).
"""

import numpy as np

import concourse.bass as bass
import concourse.tile as tile
from concourse.tile import add_dep_helper
from concourse import mybir
from concourse.bass_utils import run_bass_kernel_spmd

F32 = mybir.dt.float32
F16 = mybir.dt.float16
U8 = mybir.dt.uint8
AF = mybir.ActivationFunctionType
ALU = mybir.AluOpType

B, LQ, LK, D = 8, 2048, 2048, 256
NT = LK // 128  # 16 j-tiles (and i-tiles)
NG = 4          # i-tile groups
GS = NT // NG   # i-tiles per group
LN_EPS = 1e-5


def _fix_sync_waits(nc, max_waits: int = 1) -> int:
    """walrus here supports one sync wait per instruction; Tile emits more.
    Hoist excess waits onto same-engine NoOps inserted before the owner."""
    n = 0
    for f in nc.m.functions:
        for bb in f.blocks:
            out = []
            changed = False
            for inst in bb.instructions:
                si = inst.sync_info
                waits = list(si.on_wait) if si is not None else []
                if len(waits) > max_waits and inst.engine is not None:
                    changed = True
                    rest = waits[max_waits:]
                    for i in range(0, len(rest), max_waits):
                        nop = mybir.InstNoOp(
                            name=f"{inst.name}-syncw{n}",
                            sync_info=mybir.SyncInfo(
                                on_wait=rest[i : i + max_waits], on_update=[]
                            ),
                            bass_nofuse=True,
                            engine=inst.engine,
                        )
                        n += 1
                        out.append(nop)
                    inst.sync_info = mybir.SyncInfo(
                        on_wait=waits[:max_waits], on_update=list(si.on_update)
                    )
                out.append(inst)
            if changed:
                bb.instructions = out
    return n


def _bcast_part(ap, parts=128):
    return bass.AP(
        tensor=ap.tensor, offset=ap.offset, ap=[[0, parts]] + list(ap.ap[1:])
    )


def _bcast_mid(ap, n):
    return bass.AP(
        tensor=ap.tensor,
        offset=ap.offset,
        ap=[list(ap.ap[0]), [0, n]] + [list(a) for a in ap.ap[1:]],
    )


def build_nc():
    nc = bass.Bass("TRN2")

    q_d = nc.dram_tensor("q", [LQ, D], F32, kind="ExternalInput")
    k_d = nc.dram_tensor("k", [LK, D], F32, kind="ExternalInput")
    v_d = nc.dram_tensor("v", [LK, D], F32, kind="ExternalInput")
    mask_d = nc.dram_tensor("mask", [LQ, LK], U8, kind="ExternalInput")
    wk_d = nc.dram_tensor("wk", [1, D], F32, kind="ExternalInput")
    fcw_d = nc.dram_tensor("fcw", [D, D], F32, kind="ExternalInput")
    gamma_d = nc.dram_tensor("gamma", [1, D], F32, kind="ExternalInput")
    beta_d = nc.dram_tensor("beta", [1, D], F32, kind="ExternalInput")
    ident_d = nc.dram_tensor("ident", [128, 128], F32, kind="ExternalInput")
    pb_d = nc.dram_tensor("pb", [128, 2, 256], F32, kind="ExternalInput")

    out_d = nc.dram_tensor("out", [LQ, D], F32, kind="ExternalOutput")
    attn_d = nc.dram_tensor("attn", [LQ, LK], F32, kind="ExternalOutput")

    k_perm = k_d[:, :].rearrange("(c p two) d -> p c two d", p=128, two=2)
    v_perm = v_d[:, :].rearrange("(c p two) d -> p c two d", p=128, two=2)
    q_grp = q_d[:, :].rearrange("(g ii p) d -> g p ii d", p=128, ii=GS)
    out_grp = out_d[:, :].rearrange("(g ii p) d -> g p ii d", p=128, ii=GS)
    mask_f16 = mask_d[:, :].bitcast(F16)  # [2048, 1024]

    with tile.TileContext(nc) as tc:
        with (
            tc.tile_pool(name="const", bufs=1) as const,
            tc.tile_pool(name="stat", bufs=1) as stat,
            tc.tile_pool(name="kvp", bufs=1) as kvp,
            tc.tile_pool(name="dump", bufs=1) as dumpp,
            tc.tile_pool(name="mt", bufs=4) as mtp,
            tc.tile_pool(name="attnp", bufs=2) as attnp,
            tc.tile_pool(name="at32p", bufs=1) as at32p,
            tc.tile_pool(name="mnat", bufs=2) as mnatp,
            tc.tile_pool(name="mnat16", bufs=2) as mnatp16,
            tc.tile_pool(name="grp1", bufs=2) as grp1,
            tc.tile_pool(name="small", bufs=2) as small,
        ):
            # ------------- early loads: scalar ring first (no xbar there) --
            wk_bc = const.tile([128, D], F32)
            nc.gpsimd.dma_start(out=wk_bc, in_=_bcast_part(wk_d[0:1, :]))
            ones_row = const.tile([1, 128], F32)
            nc.vector.memset(ones_row, 1.0)
            eps_sb = const.tile([128, 1], F32)
            nc.vector.memset(eps_sb, LN_EPS)

            k_sb = kvp.tile([128, 8, 2, D], F32, tag="k")
            nc.sync.dma_start(out=k_sb, in_=k_perm)
            v_sb = kvp.tile([128, 8, 2, D], F32, tag="v")
            nc.scalar.dma_start(out=v_sb, in_=v_perm)
            fcw_sb = const.tile([128, 2, 256], F32)
            nc.scalar.dma_start(
                out=fcw_sb, in_=fcw_d[:, :].rearrange("(t p) d -> p t d", p=128)
            )
            ident = const.tile([128, 128], F32)
            nc.scalar.dma_start(out=ident, in_=ident_d[:, :])
            pb = const.tile([128, 2, 256], F32)
            nc.scalar.dma_start(out=pb, in_=pb_d[:, :, :])

            # ------------- mask transposes (exclusive xbar window) ---------
            mt_tiles = {}
            tr_inst = None
            for h in range(2):
                for c in range(8):
                    mt = mtp.tile(
                        [128, LQ // 2], F16, tag="mt", name=f"mt_{h}_{c}"
                    )
                    tr_inst = nc.sync.dma_start_transpose(
                        mt,
                        mask_f16[
                            1024 * h : 1024 * (h + 1), 128 * c : 128 * (c + 1)
                        ],
                    )
                    mt_tiles[(h, c)] = mt

            # ------------- post-window loads ------------------------------
            gamma_bc = const.tile([128, D], F32)
            nc.gpsimd.dma_start(out=gamma_bc, in_=_bcast_part(gamma_d[0:1, :]))
            beta_bc = const.tile([128, D], F32)
            nc.gpsimd.dma_start(out=beta_bc, in_=_bcast_part(beta_d[0:1, :]))

            # natural mask: even i-tiles as u8 (HWDGE sync, behind the
            # transposes in FIFO), odd i-tiles as f16 via SWDGE cast (must
            # not overlap the xbar window -> explicit dep)
            mnat_tiles = []
            for it in range(NT):
                if it % 2 == 0:
                    mnat = mnatp.tile(
                        [128, LK], U8, tag="mnat8", name=f"mnat_{it}"
                    )
                    nc.sync.dma_start(
                        out=mnat, in_=mask_d[128 * it : 128 * (it + 1), :]
                    )
                else:
                    mnat = mnatp16.tile(
                        [128, LK], F16, tag="mnat16", name=f"mnat_{it}"
                    )
                    mn_inst = nc.gpsimd.dma_start(
                        out=mnat, in_=mask_d[128 * it : 128 * (it + 1), :]
                    )
                    add_dep_helper(
                        mn_inst.ins, tr_inst.ins,
                        reason="SWDGE cast-load must not overlap xbar window",
                    )
                mnat_tiles.append(mnat)

            # ------------- persistent activations -------------------------
            sk_sb = stat.tile([128, NT], F32)
            e_perm = stat.tile([128, NT], F32)
            zinv = stat.tile([128, NT], F32)
            nzinv = stat.tile([128, NT], F32)
            ebcast = stat.tile([128, LK], F16)
            tT = stat.tile([128, NT, LQ], F16)
            evW1 = stat.tile([128, NT, 257], F16)

            # ------------- stage A: sk = k @ w_k (DVE) --------------------
            for t in range(NT):
                dump = dumpp.tile([128, D], F32)
                nc.vector.scalar_tensor_tensor(
                    out=dump,
                    in0=k_sb[:, t // 2, t % 2, :],
                    scalar=1.0,
                    in1=wk_bc,
                    op0=ALU.bypass,
                    op1=ALU.mult,
                    accum_out=sk_sb[:, t : t + 1],
                )

            # ------------- stage B: tT = (1 - m^T) fp16 -------------------
            def build_tT(h, c):
                mt_u8 = mt_tiles[(h, c)][:].bitcast(U8)
                sl = slice(1024 * h, 1024 * (h + 1))
                for b in range(2):
                    t = 2 * c + b
                    if c < 5:
                        nc.scalar.activation(
                            out=tT[:, t, sl],
                            in_=mt_u8[:, b::2],
                            func=AF.Identity,
                            bias=1.0,
                            scale=-1.0,
                        )
                    else:
                        nc.vector.tensor_scalar(
                            out=tT[:, t, sl],
                            in0=mt_u8[:, b::2],
                            scalar1=-1.0,
                            scalar2=1.0,
                            op0=ALU.mult,
                            op1=ALU.add,
                        )

            for c in range(5):
                build_tT(0, c)
            nc.scalar.activation(out=e_perm, in_=sk_sb, func=AF.Exp)

            with (
                tc.tile_pool(name="psm", bufs=2, space="PSUM") as psm,
                tc.tile_pool(name="psB", bufs=2, space="PSUM") as psB,
            ):
                # ------------- stage E: ebcast (fp16) ---------------------
                for ch in range(4):
                    ps_en = psm.tile([1, 512], F32, tag="ps")
                    for cc in range(2):
                        c = 2 * ch + cc
                        for b in range(2):
                            nc.tensor.matmul(
                                ps_en[0:1, 256 * cc : 256 * (cc + 1)],
                                lhsT=e_perm[:, 2 * c + b : 2 * c + b + 1],
                                rhs=pb[:, b, :],
                                start=(b == 0),
                                stop=(b == 1),
                            )
                    e_nat_c = small.tile([1, 512], F32, tag="enat", name=f"en_{ch}")
                    nc.scalar.copy(e_nat_c, ps_en)
                    ps_eb = psm.tile([128, 512], F32, tag="ps")
                    nc.tensor.matmul(
                        ps_eb, lhsT=ones_row, rhs=e_nat_c, start=True, stop=True
                    )
                    nc.vector.tensor_copy(
                        ebcast[:, 512 * ch : 512 * (ch + 1)], ps_eb
                    )

                # fcwT from fcw
                fcwT = const.tile([128, 2, 256], F32)
                for dt in range(2):
                    ps_t = psm.tile([128, 512], F32, tag="ps")
                    for ct in range(2):
                        nc.tensor.transpose(
                            ps_t[:, 128 * ct : 128 * (ct + 1)],
                            fcw_sb[:, ct, 128 * dt : 128 * (dt + 1)],
                            ident,
                        )
                    nc.vector.tensor_copy(fcwT[:, dt, :], ps_t[:, 0:256])

                # ------------- stage C: evW1 = [e*(v@fcw.T) | e] ----------
                for t in range(NT):
                    v_tile = v_sb[:, t // 2, t % 2, :]
                    ps_vt = psm.tile([128, 512], F32, tag="ps")
                    for dh in range(2):
                        nc.tensor.transpose(
                            ps_vt[:, 128 * dh : 128 * (dh + 1)],
                            v_tile[:, 128 * dh : 128 * (dh + 1)],
                            ident,
                        )
                    vT_sb = dumpp.tile([128, D], F32, tag="vT")
                    nc.scalar.copy(vT_sb, ps_vt[:, 0:256])
                    ps_vw = psB.tile([128, 256], F32)
                    for dt in range(2):
                        nc.tensor.matmul(
                            ps_vw,
                            lhsT=vT_sb[:, 128 * dt : 128 * (dt + 1)],
                            rhs=fcwT[:, dt, :],
                            start=(dt == 0),
                            stop=(dt == 1),
                        )
                    nc.vector.tensor_scalar_mul(
                        evW1[:, t, 0:256], ps_vw, e_perm[:, t : t + 1]
                    )
                    nc.vector.tensor_copy(
                        evW1[:, t, 256:257], e_perm[:, t : t + 1]
                    )

                # remaining tT builds
                for c in range(5, 8):
                    build_tT(0, c)
                for c in range(8):
                    build_tT(1, c)

            # ------------- stage D: t-outer accumulation ------------------
            with tc.tile_pool(name="psN", bufs=8, space="PSUM") as psN:
                for g in range(NG):
                    ps_tiles = [
                        psN.tile([128, 257], F32, tag="num", name=f"num_{g}_{i_}")
                        for i_ in range(GS)
                    ]
                    for t in range(NT):
                        for ii in range(GS):
                            it = GS * g + ii
                            nc.tensor.matmul(
                                ps_tiles[ii],
                                lhsT=tT[:, t, 128 * it : 128 * (it + 1)],
                                rhs=evW1[:, t, :],
                                start=(t == 0),
                                stop=(t == NT - 1),
                            )

                    # ---- group drain ----
                    x2_g = grp1.tile([128, GS, D], F32, tag="x2", name=f"x2_{g}")
                    for ii in range(GS):
                        it = GS * g + ii
                        zcol = zinv[:, it : it + 1]
                        nc.vector.reciprocal(zcol, ps_tiles[ii][:, 256:257])
                        nc.scalar.mul(x2_g[:, ii, :], ps_tiles[ii][:, 0:256], zcol)
                    nc.vector.tensor_scalar_mul(
                        nzinv[:, GS * g : GS * (g + 1)],
                        zinv[:, GS * g : GS * (g + 1)],
                        -1.0,
                    )

                    # attn = (m==0)*zinv (*) e  : u8 tiles via ACT, f16 via DVE
                    for ii in range(GS):
                        it = GS * g + ii
                        zcol = zinv[:, it : it + 1]
                        uz = attnp.tile(
                            [128, LK], F16, tag="uz", name=f"uz_{g}_{ii}"
                        )
                        if it % 2 == 0:
                            nc.scalar.activation(
                                out=uz,
                                in_=mnat_tiles[it],
                                func=AF.Identity,
                                bias=zcol,
                                scale=nzinv[:, it : it + 1],
                            )
                        else:
                            nc.vector.tensor_scalar(
                                out=uz,
                                in0=mnat_tiles[it],
                                scalar1=0.0,
                                scalar2=zcol,
                                op0=ALU.is_equal,
                                op1=ALU.mult,
                            )
                        if it % 2 == 0:
                            at = at32p.tile(
                                [128, LK], F32, tag="at32", name=f"at_{g}_{ii}"
                            )
                            nc.vector.tensor_tensor(at, uz, ebcast, ALU.mult)
                            eng = nc.sync if ii % 2 == 0 else nc.scalar
                            eng.dma_start(
                                out=attn_d[128 * it : 128 * (it + 1), :], in_=at
                            )
                        else:
                            at = attnp.tile(
                                [128, LK], F16, tag="at16", name=f"at_{g}_{ii}"
                            )
                            nc.vector.tensor_tensor(at, uz, ebcast, ALU.mult)
                            nc.gpsimd.dma_start(
                                out=attn_d[128 * it : 128 * (it + 1), :], in_=at
                            )

                    # residual + layernorm (batched over the group)
                    q_g = grp1.tile([128, GS, D], F32, tag="qg", name=f"q_{g}")
                    nc.scalar.dma_start(out=q_g, in_=q_grp[g])
                    nc.gpsimd.tensor_tensor(x2_g, x2_g, q_g, ALU.add)
                    stats_g = small.tile(
                        [128, GS, 6], F32, tag="stats", name=f"st_{g}"
                    )
                    for ii in range(GS):
                        nc.vector.bn_stats(stats_g[:, ii, :], x2_g[:, ii, :])
                    mv_g = small.tile([128, GS, 2], F32, tag="mv", name=f"mv_{g}")
                    for ii in range(GS):
                        nc.vector.bn_aggr(mv_g[:, ii, :], stats_g[:, ii, :])
                    rstd_g = small.tile([128, GS], F32, tag="rstd", name=f"rs_{g}")
                    nc.scalar.activation(
                        out=rstd_g, in_=mv_g[:, :, 1], func=AF.Sqrt,
                        bias=eps_sb, scale=1.0,
                    )
                    nc.vector.reciprocal(rstd_g, rstd_g)
                    nmr_g = small.tile([128, GS], F32, tag="nmr", name=f"nm_{g}")
                    nc.vector.scalar_tensor_tensor(
                        out=nmr_g,
                        in0=mv_g[:, :, 0],
                        scalar=-1.0,
                        in1=rstd_g,
                        op0=ALU.mult,
                        op1=ALU.mult,
                    )
                    for ii in range(GS):
                        nc.scalar.activation(
                            out=x2_g[:, ii, :],
                            in_=x2_g[:, ii, :],
                            func=AF.Identity,
                            scale=rstd_g[:, ii : ii + 1],
                            bias=nmr_g[:, ii : ii + 1],
                        )
                    nc.gpsimd.tensor_tensor(
                        x2_g, x2_g, _bcast_mid(gamma_bc[:, :], GS), ALU.mult
                    )
                    nc.gpsimd.tensor_tensor(
                        x2_g, x2_g, _bcast_mid(beta_bc[:, :], GS), ALU.add
                    )
                    nc.scalar.dma_start(out=out_grp[g], in_=x2_g)

    _fix_sync_waits(nc)
    return nc


_NC = None
_LAST_IN_MAPS = None


def _get_nc():
    global _NC
    if _NC is None:
        _NC = build_nc()
    return _NC


def kernel(q, k, v, shared_attn, fc_w, ln_gamma, ln_beta, mask):
    q = np.asarray(q)
    k = np.asarray(k)
    v = np.asarray(v)
    shared_attn = np.asarray(shared_attn)
    fc_w = np.asarray(fc_w)
    ln_gamma = np.asarray(ln_gamma)
    ln_beta = np.asarray(ln_beta)
    mask_u8 = np.asarray(mask).view(np.uint8)

    wk = np.ascontiguousarray(shared_attn[:, D:])
    gamma = np.ascontiguousarray(ln_gamma.reshape(1, D)).astype(np.float32)
    beta = np.ascontiguousarray(ln_beta.reshape(1, D)).astype(np.float32)
    ident = np.eye(128, dtype=np.float32)
    pb = np.zeros((128, 2, 256), dtype=np.float32)
    p_idx = np.arange(128)
    for b in range(2):
        pb[p_idx, b, 2 * p_idx + b] = 1.0

    nc = _get_nc()
    in_maps = []
    for b_i in range(B):
        in_maps.append(
            {
                "q": np.ascontiguousarray(q[b_i]),
                "k": np.ascontiguousarray(k[b_i]),
                "v": np.ascontiguousarray(v[b_i]),
                "mask": np.ascontiguousarray(mask_u8[b_i]),
                "wk": wk,
                "fcw": np.ascontiguousarray(fc_w),
                "gamma": gamma,
                "beta": beta,
                "ident": ident,
                "pb": pb,
            }
        )
    global _LAST_IN_MAPS
    _LAST_IN_MAPS = in_maps
    res = run_bass_kernel_spmd(nc, in_maps, core_ids=list(range(B)))
    out = np.stack([res.results[c]["out"] for c in range(B)])
    attn = np.stack([res.results[c]["attn"] for c in range(B)])
    return out, attn


# revision 34
# speedup vs baseline: 1.0530x; 1.0530x over previous
"""Trainium2 Bass kernel for nn_Attention_4380866642117.

Math: the reference computes additive-score attention
    score[b,i,j] = q[b,i].w_q + k[b,j].w_k ; masked (mask==True -> -1e10)
    attn = softmax_j(score); out = LN(attn @ v @ fc_w.T + q)
The additive score's q-term is constant along the softmax axis and cancels;
masked logits (-1e10) underflow to exactly 0 in the f32 softmax.  With
e_j = exp(k[b,j].w_k):
    attn[b,i,j] = (1-m[b,i,j]) * e_j / Z_i,  Z_i = sum_j (1-m) e_j
    out[b,i,:]  = LN( (1/Z_i) sum_j (1-m) e_j (v@fc_w.T)[j,:] + q[b,i,:] )
so attention+fc collapse to one masked matmul against fc-pretransformed,
e-weighted values, plus a rank-1 masked elementwise product for attn.

Distribution: data-parallel over batch (8 batches -> 8 NeuronCores).

The mask enters the PE with j on partitions (contraction dim) by loading it
through the DMA xbar transpose viewing byte pairs as one fp16 element; the
resulting row pairing j = 256c + 2p + b is absorbed by loading k/v rows with
the same permuted access pattern (contraction is order-invariant).
"""

import numpy as np

import concourse.bass as bass
import concourse.tile as tile
from concourse.tile import add_dep_helper
from concourse import mybir
from concourse.bass_utils import run_bass_kernel_spmd

F32 = mybir.dt.float32
F16 = mybir.dt.float16
U8 = mybir.dt.uint8
AF = mybir.ActivationFunctionType
ALU = mybir.AluOpType

B, LQ, LK, D = 8, 2048, 2048, 256
NT = LK // 128  # 16 j-tiles (and i-tiles)
NG = 4          # i-tile groups
GS = NT // NG   # i-tiles per group
LN_EPS = 1e-5


def _fix_sync_waits(nc, max_waits: int = 1) -> int:
    """walrus here supports one sync wait per instruction; Tile emits more.
    Hoist excess waits onto same-engine NoOps inserted before the owner."""
    n = 0
    for f in nc.m.functions:
        for bb in f.blocks:
            out = []
            changed = False
            for inst in bb.instructions:
                si = inst.sync_info
                waits = list(si.on_wait) if si is not None else []
                if len(waits) > max_waits and inst.engine is not None:
                    changed = True
                    rest = waits[max_waits:]
                    for i in range(0, len(rest), max_waits):
                        nop = mybir.InstNoOp(
                            name=f"{inst.name}-syncw{n}",
                            sync_info=mybir.SyncInfo(
                                on_wait=rest[i : i + max_waits], on_update=[]
                            ),
                            bass_nofuse=True,
                            engine=inst.engine,
                        )
                        n += 1
                        out.append(nop)
                    inst.sync_info = mybir.SyncInfo(
                        on_wait=waits[:max_waits], on_update=list(si.on_update)
                    )
                out.append(inst)
            if changed:
                bb.instructions = out
    return n


def _bcast_part(ap, parts=128):
    return bass.AP(
        tensor=ap.tensor, offset=ap.offset, ap=[[0, parts]] + list(ap.ap[1:])
    )


def _bcast_mid(ap, n):
    return bass.AP(
        tensor=ap.tensor,
        offset=ap.offset,
        ap=[list(ap.ap[0]), [0, n]] + [list(a) for a in ap.ap[1:]],
    )


def build_nc():
    nc = bass.Bass("TRN2")

    q_d = nc.dram_tensor("q", [LQ, D], F32, kind="ExternalInput")
    k_d = nc.dram_tensor("k", [LK, D], F32, kind="ExternalInput")
    v_d = nc.dram_tensor("v", [LK, D], F32, kind="ExternalInput")
    mask_d = nc.dram_tensor("mask", [LQ, LK], U8, kind="ExternalInput")
    wk_d = nc.dram_tensor("wk", [1, D], F32, kind="ExternalInput")
    fcw_d = nc.dram_tensor("fcw", [D, D], F32, kind="ExternalInput")
    gamma_d = nc.dram_tensor("gamma", [1, D], F32, kind="ExternalInput")
    beta_d = nc.dram_tensor("beta", [1, D], F32, kind="ExternalInput")
    ident_d = nc.dram_tensor("ident", [128, 128], F32, kind="ExternalInput")
    pb_d = nc.dram_tensor("pb", [128, 2, 256], F32, kind="ExternalInput")

    out_d = nc.dram_tensor("out", [LQ, D], F32, kind="ExternalOutput")
    attn_d = nc.dram_tensor("attn", [LQ, LK], F32, kind="ExternalOutput")

    k_perm = k_d[:, :].rearrange("(c p two) d -> p c two d", p=128, two=2)
    v_perm = v_d[:, :].rearrange("(c p two) d -> p c two d", p=128, two=2)
    q_grp = q_d[:, :].rearrange("(g ii p) d -> g p ii d", p=128, ii=GS)
    out_grp = out_d[:, :].rearrange("(g ii p) d -> g p ii d", p=128, ii=GS)
    mask_f16 = mask_d[:, :].bitcast(F16)  # [2048, 1024]

    with tile.TileContext(nc) as tc:
        with (
            tc.tile_pool(name="const", bufs=1) as const,
            tc.tile_pool(name="stat", bufs=1) as stat,
            tc.tile_pool(name="dump", bufs=1) as dumpp,
            tc.tile_pool(name="mt", bufs=4) as mtp,
            tc.tile_pool(name="attnp", bufs=2) as attnp,
            tc.tile_pool(name="at32p", bufs=1) as at32p,
            tc.tile_pool(name="mnat", bufs=2) as mnatp,
            tc.tile_pool(name="mnat16", bufs=2) as mnatp16,
            tc.tile_pool(name="grp1", bufs=2) as grp1,
            tc.tile_pool(name="small", bufs=2) as small,
        ):
            # ------------- early loads: scalar ring first (no xbar there) --
            wk_bc = const.tile([128, D], F32)
            nc.gpsimd.dma_start(out=wk_bc, in_=_bcast_part(wk_d[0:1, :]))
            ones_row = const.tile([1, 128], F32)
            nc.vector.memset(ones_row, 1.0)
            eps_sb = const.tile([128, 1], F32)
            nc.vector.memset(eps_sb, LN_EPS)

            kvp_ctx = tc.tile_pool(name="kvp", bufs=1)
            kvp = kvp_ctx.__enter__()
            k_sb = kvp.tile([128, 8, 2, D], F32, tag="k")
            nc.sync.dma_start(out=k_sb, in_=k_perm)
            v_sb = kvp.tile([128, 8, 2, D], F32, tag="v")
            nc.scalar.dma_start(out=v_sb, in_=v_perm)
            fcw_sb = const.tile([128, 2, 256], F32)
            nc.scalar.dma_start(
                out=fcw_sb, in_=fcw_d[:, :].rearrange("(t p) d -> p t d", p=128)
            )
            ident = const.tile([128, 128], F32)
            nc.scalar.dma_start(out=ident, in_=ident_d[:, :])
            pb = const.tile([128, 2, 256], F32)
            nc.scalar.dma_start(out=pb, in_=pb_d[:, :, :])

            # ------------- mask transposes (exclusive xbar window) ---------
            mt_tiles = {}
            tr_inst = None
            for h in range(2):
                for c in range(8):
                    mt = mtp.tile(
                        [128, LQ // 2], F16, tag="mt", name=f"mt_{h}_{c}"
                    )
                    tr_inst = nc.sync.dma_start_transpose(
                        mt,
                        mask_f16[
                            1024 * h : 1024 * (h + 1), 128 * c : 128 * (c + 1)
                        ],
                    )
                    mt_tiles[(h, c)] = mt

            # ------------- post-window loads ------------------------------
            gamma_bc = const.tile([128, D], F32)
            nc.gpsimd.dma_start(out=gamma_bc, in_=_bcast_part(gamma_d[0:1, :]))
            beta_bc = const.tile([128, D], F32)
            nc.gpsimd.dma_start(out=beta_bc, in_=_bcast_part(beta_d[0:1, :]))

            # natural mask: even i-tiles as u8 (HWDGE sync, behind the
            # transposes in FIFO), odd i-tiles as f16 via SWDGE cast (must
            # not overlap the xbar window -> explicit dep)
            mnat_tiles = []
            for it in range(NT):
                if it % 2 == 0:
                    mnat = mnatp.tile(
                        [128, LK], U8, tag="mnat8", name=f"mnat_{it}"
                    )
                    nc.sync.dma_start(
                        out=mnat, in_=mask_d[128 * it : 128 * (it + 1), :]
                    )
                else:
                    mnat = mnatp16.tile(
                        [128, LK], F16, tag="mnat16", name=f"mnat_{it}"
                    )
                    mn_inst = nc.gpsimd.dma_start(
                        out=mnat, in_=mask_d[128 * it : 128 * (it + 1), :]
                    )
                    add_dep_helper(
                        mn_inst.ins, tr_inst.ins,
                        reason="SWDGE cast-load must not overlap xbar window",
                    )
                mnat_tiles.append(mnat)

            # ------------- persistent activations -------------------------
            sk_sb = stat.tile([128, NT], F32)
            e_perm = stat.tile([128, NT], F32)
            zinv = stat.tile([128, NT], F32)
            nzinv = stat.tile([128, NT], F32)
            ebcast = stat.tile([128, LK], F16)
            tT = stat.tile([128, NT, LQ], F16)
            evW1 = stat.tile([128, NT, 257], F16)

            # ------------- stage A: sk = k @ w_k (DVE) --------------------
            for t in range(NT):
                dump = dumpp.tile([128, D], F32)
                nc.vector.scalar_tensor_tensor(
                    out=dump,
                    in0=k_sb[:, t // 2, t % 2, :],
                    scalar=1.0,
                    in1=wk_bc,
                    op0=ALU.bypass,
                    op1=ALU.mult,
                    accum_out=sk_sb[:, t : t + 1],
                )

            # ------------- stage B: tT = (1 - m^T) fp16 -------------------
            def build_tT(h, c):
                mt_u8 = mt_tiles[(h, c)][:].bitcast(U8)
                sl = slice(1024 * h, 1024 * (h + 1))
                for b in range(2):
                    t = 2 * c + b
                    if c < 5:
                        nc.scalar.activation(
                            out=tT[:, t, sl],
                            in_=mt_u8[:, b::2],
                            func=AF.Identity,
                            bias=1.0,
                            scale=-1.0,
                        )
                    else:
                        nc.vector.tensor_scalar(
                            out=tT[:, t, sl],
                            in0=mt_u8[:, b::2],
                            scalar1=-1.0,
                            scalar2=1.0,
                            op0=ALU.mult,
                            op1=ALU.add,
                        )

            for c in range(5):
                build_tT(0, c)
            nc.scalar.activation(out=e_perm, in_=sk_sb, func=AF.Exp)

            with (
                tc.tile_pool(name="psm", bufs=2, space="PSUM") as psm,
                tc.tile_pool(name="psB", bufs=2, space="PSUM") as psB,
            ):
                # ------------- stage E: ebcast (fp16) ---------------------
                for ch in range(4):
                    ps_en = psm.tile([1, 512], F32, tag="ps")
                    for cc in range(2):
                        c = 2 * ch + cc
                        for b in range(2):
                            nc.tensor.matmul(
                                ps_en[0:1, 256 * cc : 256 * (cc + 1)],
                                lhsT=e_perm[:, 2 * c + b : 2 * c + b + 1],
                                rhs=pb[:, b, :],
                                start=(b == 0),
                                stop=(b == 1),
                            )
                    e_nat_c = small.tile([1, 512], F32, tag="enat", name=f"en_{ch}")
                    nc.scalar.copy(e_nat_c, ps_en)
                    ps_eb = psm.tile([128, 512], F32, tag="ps")
                    nc.tensor.matmul(
                        ps_eb, lhsT=ones_row, rhs=e_nat_c, start=True, stop=True
                    )
                    nc.vector.tensor_copy(
                        ebcast[:, 512 * ch : 512 * (ch + 1)], ps_eb
                    )

                # fcwT from fcw
                fcwT = const.tile([128, 2, 256], F32)
                for dt in range(2):
                    ps_t = psm.tile([128, 512], F32, tag="ps")
                    for ct in range(2):
                        nc.tensor.transpose(
                            ps_t[:, 128 * ct : 128 * (ct + 1)],
                            fcw_sb[:, ct, 128 * dt : 128 * (dt + 1)],
                            ident,
                        )
                    nc.vector.tensor_copy(fcwT[:, dt, :], ps_t[:, 0:256])

                # ------------- stage C: evW1 = [e*(v@fcw.T) | e] ----------
                for t in range(NT):
                    v_tile = v_sb[:, t // 2, t % 2, :]
                    ps_vt = psm.tile([128, 512], F32, tag="ps")
                    for dh in range(2):
                        nc.tensor.transpose(
                            ps_vt[:, 128 * dh : 128 * (dh + 1)],
                            v_tile[:, 128 * dh : 128 * (dh + 1)],
                            ident,
                        )
                    vT_sb = dumpp.tile([128, D], F32, tag="vT")
                    nc.scalar.copy(vT_sb, ps_vt[:, 0:256])
                    ps_vw = psB.tile([128, 256], F32)
                    for dt in range(2):
                        nc.tensor.matmul(
                            ps_vw,
                            lhsT=vT_sb[:, 128 * dt : 128 * (dt + 1)],
                            rhs=fcwT[:, dt, :],
                            start=(dt == 0),
                            stop=(dt == 1),
                        )
                    nc.vector.tensor_scalar_mul(
                        evW1[:, t, 0:256], ps_vw, e_perm[:, t : t + 1]
                    )
                    nc.vector.tensor_copy(
                        evW1[:, t, 256:257], e_perm[:, t : t + 1]
                    )

                # remaining tT builds
                for c in range(5, 8):
                    build_tT(0, c)
                for c in range(8):
                    build_tT(1, c)

            # ------------- stage D: t-outer accumulation ------------------
            kvp_ctx.__exit__(None, None, None)
            with (
                tc.tile_pool(name="psN", bufs=8, space="PSUM") as psN,
                tc.tile_pool(name="x2p", bufs=4) as x2p,
            ):
                x2_tiles = []
                # D1: all matmuls + immediate PSUM release (recip + y-scale)
                for g in range(NG):
                    ps_tiles = [
                        psN.tile([128, 257], F32, tag="num", name=f"num_{g}_{i_}")
                        for i_ in range(GS)
                    ]
                    for t in range(NT):
                        for ii in range(GS):
                            it = GS * g + ii
                            nc.tensor.matmul(
                                ps_tiles[ii],
                                lhsT=tT[:, t, 128 * it : 128 * (it + 1)],
                                rhs=evW1[:, t, :],
                                start=(t == 0),
                                stop=(t == NT - 1),
                            )
                    x2_g = x2p.tile([128, GS, D], F32, tag="x2", name=f"x2_{g}")
                    x2_tiles.append(x2_g)
                    for ii in range(GS):
                        it = GS * g + ii
                        zcol = zinv[:, it : it + 1]
                        nc.vector.reciprocal(zcol, ps_tiles[ii][:, 256:257])
                        nc.scalar.mul(x2_g[:, ii, :], ps_tiles[ii][:, 0:256], zcol)

                # D2: attn + layernorm drains
                for g in range(NG):
                    x2_g = x2_tiles[g]
                    nc.vector.tensor_scalar_mul(
                        nzinv[:, GS * g : GS * (g + 1)],
                        zinv[:, GS * g : GS * (g + 1)],
                        -1.0,
                    )

                    # attn = (m==0)*zinv (*) e  : u8 tiles via ACT, f16 via DVE
                    for ii in range(GS):
                        it = GS * g + ii
                        zcol = zinv[:, it : it + 1]
                        uz = attnp.tile(
                            [128, LK], F16, tag="uz", name=f"uz_{g}_{ii}"
                        )
                        if it % 2 == 0:
                            nc.scalar.activation(
                                out=uz,
                                in_=mnat_tiles[it],
                                func=AF.Identity,
                                bias=zcol,
                                scale=nzinv[:, it : it + 1],
                            )
                        else:
                            nc.vector.tensor_scalar(
                                out=uz,
                                in0=mnat_tiles[it],
                                scalar1=0.0,
                                scalar2=zcol,
                                op0=ALU.is_equal,
                                op1=ALU.mult,
                            )
                        if it % 2 == 0:
                            at = at32p.tile(
                                [128, LK], F32, tag="at32", name=f"at_{g}_{ii}"
                            )
                            nc.vector.tensor_tensor(at, uz, ebcast, ALU.mult)
                            eng = nc.sync if ii % 2 == 0 else nc.scalar
                            eng.dma_start(
                                out=attn_d[128 * it : 128 * (it + 1), :], in_=at
                            )
                        else:
                            at = attnp.tile(
                                [128, LK], F16, tag="at16", name=f"at_{g}_{ii}"
                            )
                            nc.vector.tensor_tensor(at, uz, ebcast, ALU.mult)
                            nc.gpsimd.dma_start(
                                out=attn_d[128 * it : 128 * (it + 1), :], in_=at
                            )

                    # residual + layernorm (batched over the group)
                    q_g = grp1.tile([128, GS, D], F32, tag="qg", name=f"q_{g}")
                    nc.scalar.dma_start(out=q_g, in_=q_grp[g])
                    nc.gpsimd.tensor_tensor(x2_g, x2_g, q_g, ALU.add)
                    stats_g = small.tile(
                        [128, GS, 6], F32, tag="stats", name=f"st_{g}"
                    )
                    for ii in range(GS):
                        nc.vector.bn_stats(stats_g[:, ii, :], x2_g[:, ii, :])
                    mv_g = small.tile([128, GS, 2], F32, tag="mv", name=f"mv_{g}")
                    for ii in range(GS):
                        nc.vector.bn_aggr(mv_g[:, ii, :], stats_g[:, ii, :])
                    rstd_g = small.tile([128, GS], F32, tag="rstd", name=f"rs_{g}")
                    nc.scalar.activation(
                        out=rstd_g, in_=mv_g[:, :, 1], func=AF.Sqrt,
                        bias=eps_sb, scale=1.0,
                    )
                    nc.vector.reciprocal(rstd_g, rstd_g)
                    nmr_g = small.tile([128, GS], F32, tag="nmr", name=f"nm_{g}")
                    nc.vector.scalar_tensor_tensor(
                        out=nmr_g,
                        in0=mv_g[:, :, 0],
                        scalar=-1.0,
                        in1=rstd_g,
                        op0=ALU.mult,
                        op1=ALU.mult,
                    )
                    for ii in range(GS):
                        nc.scalar.activation(
                            out=x2_g[:, ii, :],
                            in_=x2_g[:, ii, :],
                            func=AF.Identity,
                            scale=rstd_g[:, ii : ii + 1],
                            bias=nmr_g[:, ii : ii + 1],
                        )
                    nc.gpsimd.tensor_tensor(
                        x2_g, x2_g, _bcast_mid(gamma_bc[:, :], GS), ALU.mult
                    )
                    nc.gpsimd.tensor_tensor(
                        x2_g, x2_g, _bcast_mid(beta_bc[:, :], GS), ALU.add
                    )
                    nc.scalar.dma_start(out=out_grp[g], in_=x2_g)

    _fix_sync_waits(nc)
    return nc


_NC = None
_LAST_IN_MAPS = None


def _get_nc():
    global _NC
    if _NC is None:
        _NC = build_nc()
    return _NC


def kernel(q, k, v, shared_attn, fc_w, ln_gamma, ln_beta, mask):
    q = np.asarray(q)
    k = np.asarray(k)
    v = np.asarray(v)
    shared_attn = np.asarray(shared_attn)
    fc_w = np.asarray(fc_w)
    ln_gamma = np.asarray(ln_gamma)
    ln_beta = np.asarray(ln_beta)
    mask_u8 = np.asarray(mask).view(np.uint8)

    wk = np.ascontiguousarray(shared_attn[:, D:])
    gamma = np.ascontiguousarray(ln_gamma.reshape(1, D)).astype(np.float32)
    beta = np.ascontiguousarray(ln_beta.reshape(1, D)).astype(np.float32)
    ident = np.eye(128, dtype=np.float32)
    pb = np.zeros((128, 2, 256), dtype=np.float32)
    p_idx = np.arange(128)
    for b in range(2):
        pb[p_idx, b, 2 * p_idx + b] = 1.0

    nc = _get_nc()
    in_maps = []
    for b_i in range(B):
        in_maps.append(
            {
                "q": np.ascontiguousarray(q[b_i]),
                "k": np.ascontiguousarray(k[b_i]),
                "v": np.ascontiguousarray(v[b_i]),
                "mask": np.ascontiguousarray(mask_u8[b_i]),
                "wk": wk,
                "fcw": np.ascontiguousarray(fc_w),
                "gamma": gamma,
                "beta": beta,
                "ident": ident,
                "pb": pb,
            }
        )
    global _LAST_IN_MAPS
    _LAST_IN_MAPS = in_maps
    res = run_bass_kernel_spmd(nc, in_maps, core_ids=list(range(B)))
    out = np.stack([res.results[c]["out"] for c in range(B)])
    attn = np.stack([res.results[c]["attn"] for c in range(B)])
    return out, attn
